# revision 7
# baseline (speedup 1.0000x reference)
"""AttentionBlock kernel for TRN2, 8 NeuronCores, data-parallel over batch.

Reference computation (per batch element, c=512 channels, n=1024 spatial):
  GroupNorm(32 groups) -> qkv 1x1 conv -> 8-head attention (ch=64) -> proj -> residual.

Sharding: batch 16 -> 2 per core. Weights replicated. No collectives.

Key design decisions:
  - All big matmuls in bf16 (PE full rate), fp32 PSUM accumulation.
  - Scores computed TRANSPOSED: S^T[m,n] = k^T q via lhsT=k, rhs=q, so that
    exp(S^T) chunks can feed the attn@v matmul directly as the moving operand
    (contraction over m on partitions) -- no transposes anywhere.
  - Softmax: no max-subtraction (scores ~ N(0,1) after norm; exp is safe),
    exp fused with the 1/sqrt(ch) scale on ScalarE (PSUM->SBUF bf16).
  - Row sums come FREE from the attn@v matmul by appending a ones column to
    the v^T stationary operand (M=65; row 64 of the output = sum_m exp).
  - Normalization (x 1/s[n]) applied to the 64-row raw attn output before
    proj: reciprocal (DVE) -> broadcast to 64 partitions via a K=1 matmul
    with ones -> multiply. v-bias and proj bias folded host-side into an
    effective proj bias (valid because sum_m softmax = 1).
  - GroupNorm stats: x reloaded as [groups, 16*1024] (contiguous view),
    bn_stats/bn_aggr, then mean/rstd bounced through a DRAM scratch tile to
    remap group stats -> per-channel partitions (DMA gather with step-0
    partition broadcast).
"""

import sys

for _p in ("/opt/trn_rl_repo", "/opt/pypackages"):
    if _p not in sys.path:
        sys.path.append(_p)

import numpy as np
import ml_dtypes

import concourse.bass as bass
import concourse.bacc as bacc
import concourse.mybir as mybir
import concourse.tile as tile
from concourse.bass_utils import run_bass_kernel_spmd

F32 = mybir.dt.float32
BF16 = mybir.dt.bfloat16

B, C, H, W = 16, 512, 32, 32
N = H * W                      # 1024 spatial positions
NCORES = 8
BL = B // NCORES               # 2 batch elements per core
GROUPS = 32
GSIZE = C // GROUPS            # 16 channels per group
HEADS = 8
CH = C // HEADS                # 64
EPS = 1e-5
CT = C // 128                  # 4 channel tiles
NJ = N // 128                  # 8 m-chunks (key/value positions)
SCALE = CH ** -0.5


def build_nc(qk_bias_zero: bool, debug: bool = False) -> bass.Bass:
    nc = bacc.Bacc()

    x_ext = nc.declare_dram_parameter("x", [BL, C, N], F32, isOutput=False)
    wqkT_ext = nc.declare_dram_parameter("wqkT", [C, 2 * C], BF16, isOutput=False)
    wvT_ext = nc.declare_dram_parameter("wvT", [C, C], BF16, isOutput=False)
    wpT_ext = nc.declare_dram_parameter("wpT", [C, C], BF16, isOutput=False)
    gnw_ext = nc.declare_dram_parameter("gnw", [128, CT], F32, isOutput=False)
    gnb_ext = nc.declare_dram_parameter("gnb", [128, CT], F32, isOutput=False)
    qkb_ext = nc.declare_dram_parameter("qkb", [128, 8], F32, isOutput=False)
    pb_ext = nc.declare_dram_parameter("pbias", [128, CT], F32, isOutput=False)
    ind_ext = nc.declare_dram_parameter("ind", [8, 128], F32, isOutput=False)
    out_ext = nc.declare_dram_parameter("out", [BL, C, N], F32, isOutput=True)
    if debug:
        dbg_ab = nc.declare_dram_parameter("dbg_ab", [128, BL * CT, 2], F32, isOutput=True)
        dbg_xn = nc.declare_dram_parameter("dbg_xn", [BL, C, N], BF16, isOutput=True)
        dbg_qk = nc.declare_dram_parameter("dbg_qk", [BL, 8, 128, N], BF16, isOutput=True)
        dbg_vt = nc.declare_dram_parameter("dbg_vt", [BL, NJ, 128, HEADS, CH + 1], BF16, isOutput=True)
        dbg_es = nc.declare_dram_parameter("dbg_es", [BL, NJ, 128, N], BF16, isOutput=True)
        dbg_at = nc.declare_dram_parameter("dbg_at", [BL, CT, 128, N], BF16, isOutput=True)

    with tile.TileContext(nc) as tc:
        with (
            tc.tile_pool(name="singles", bufs=1) as singles,
            tc.tile_pool(name="dram", bufs=1, space="DRAM") as dram_pool,
            tc.tile_pool(name="xb", bufs=5) as xb_pool,
            tc.tile_pool(name="xn", bufs=5) as xn_pool,
            tc.tile_pool(name="qk", bufs=9) as qk_pool,
            tc.tile_pool(name="vt", bufs=9) as vt_pool,
            tc.tile_pool(name="es", bufs=3) as es_pool,
            tc.tile_pool(name="at", bufs=5) as at_pool,
            tc.tile_pool(name="ath", bufs=3) as ath_pool,
            tc.tile_pool(name="osb", bufs=3) as osb_pool,
            tc.tile_pool(name="small", bufs=4) as small_pool,
            tc.tile_pool(name="rsb", bufs=2) as rsb_pool,
            tc.tile_pool(name="psA", bufs=2, space="PSUM") as psA,
            tc.tile_pool(name="psAcc", bufs=2, space="PSUM") as psAcc,
        ):
            # ---- static weights / constants ----
            wqkT_sb = singles.tile([128, CT, 2 * C], BF16)
            nc.sync.dma_start(
                out=wqkT_sb, in_=wqkT_ext[:].rearrange("(t p) o -> p t o", p=128)
            )
            wvT_sb = singles.tile([128, CT, C], BF16)
            nc.sync.dma_start(
                out=wvT_sb, in_=wvT_ext[:].rearrange("(t p) o -> p t o", p=128)
            )
            wpT_sb = singles.tile([128, CT, C], BF16)
            nc.sync.dma_start(
                out=wpT_sb, in_=wpT_ext[:].rearrange("(t p) o -> p t o", p=128)
            )
            gnw_sb = singles.tile([128, CT], F32)
            nc.sync.dma_start(out=gnw_sb, in_=gnw_ext[:])
            gnb_sb = singles.tile([128, CT], F32)
            nc.sync.dma_start(out=gnb_sb, in_=gnb_ext[:])
            qkb_sb = singles.tile([128, 8], F32)
            nc.sync.dma_start(out=qkb_sb, in_=qkb_ext[:])
            pb_sb = singles.tile([128, CT], F32)
            nc.sync.dma_start(out=pb_sb, in_=pb_ext[:])
            ind_sb = singles.tile([8, 128], F32)
            nc.sync.dma_start(out=ind_sb, in_=ind_ext[:])

            # ones row at partition 64 (same partition as the attn@v sums row)
            ones64 = singles.tile([65, CH], F32)
            nc.vector.memset(ones64[64:65, :], 1.0)
            eps_sb = singles.tile([2 * GROUPS, 1], F32)
            nc.vector.memset(eps_sb, EPS)

            # ---- phase 0: groupnorm statistics for both batch elements ----
            # xg[p, :] = x[b, g*16:(g+1)*16, :] flattened, p = b*32+g
            xg = singles.tile([2 * GROUPS, GSIZE * N], F32)
            for b in range(BL):
                nc.sync.dma_start(
                    out=xg[b * GROUPS:(b + 1) * GROUPS, :],
                    in_=x_ext[b].rearrange("(g f) n -> g (f n)", g=GROUPS),
                )
            nchunk = (GSIZE * N) // 512
            stats = singles.tile([2 * GROUPS, nchunk, 6], F32)
            for i in range(nchunk):
                nc.vector.bn_stats(
                    out=stats[:, i, :], in_=xg[:, i * 512:(i + 1) * 512]
                )
            mv = singles.tile([2 * GROUPS, 2], F32)
            nc.vector.bn_aggr(out=mv, in_=stats)
            # mv[:,1] = rstd = 1/sqrt(var+eps)
            nc.scalar.activation(
                out=mv[:, 1:2], in_=mv[:, 1:2],
                func=mybir.ActivationFunctionType.Sqrt,
                bias=eps_sb, scale=1.0,
            )
            nc.vector.reciprocal(out=mv[:, 1:2], in_=mv[:, 1:2])

            # bounce through DRAM to remap [b*32+g, 2] -> per-channel [128, 2]
            scratch = dram_pool.tile([2 * GROUPS, 2], F32)
            nc.sync.dma_start(out=scratch, in_=mv)

            # per (b, ctile): A = rstd*w, Bc = b - mean*A  (per-channel affine)
            ab_sb = singles.tile([128, BL * CT, 2], F32)
            for b in range(BL):
                for t in range(CT):
                    bt = b * CT + t
                    msrc = small_pool.tile([8, 2], F32, tag="msrc")
                    base = scratch[0:1, 0:1]
                    gather = bass.AP(
                        tensor=base.tensor,
                        offset=base.offset + (b * GROUPS + t * 8) * 2,
                        ap=[[2, 8], [1, 2]],
                    )
                    nc.sync.dma_start(out=msrc, in_=gather)
                    # broadcast group stats to the 128 channel partitions
                    # via a K=8 indicator matmul (step-0 partition DMA
                    # broadcasts do not replicate)
                    bc_ps = psA.tile([128, 2], F32, tag="big")
                    nc.tensor.matmul(
                        out=bc_ps, lhsT=ind_sb, rhs=msrc,
                        start=True, stop=True,
                    )
                    # A = rstd * w
                    nc.vector.tensor_mul(
                        out=ab_sb[:, bt, 0:1], in0=bc_ps[:, 1:2],
                        in1=gnw_sb[:, t:t + 1],
                    )
                    # B = gnb - mean*A
                    tmpb = small_pool.tile([128, 1], F32, tag="tmpb")
                    nc.vector.tensor_mul(
                        out=tmpb, in0=bc_ps[:, 0:1], in1=ab_sb[:, bt, 0:1]
                    )
                    nc.vector.tensor_sub(
                        out=ab_sb[:, bt, 1:2], in0=gnb_sb[:, t:t + 1], in1=tmpb
                    )

            if debug:
                nc.sync.dma_start(out=dbg_ab[:], in_=ab_sb)

            # ---- phase 1: per batch element ----
            for b in range(BL):
                # load x tiles (kept fp32 for the residual), normalize -> bf16
                xb = []
                xn = []
                for t in range(CT):
                    bt = b * CT + t
                    xt = xb_pool.tile([128, N], F32, tag="xb")
                    nc.sync.dma_start(out=xt, in_=x_ext[b, t * 128:(t + 1) * 128, :])
                    xnt = xn_pool.tile([128, N], BF16, tag="xn")
                    # xn = x*A + B
                    nc.vector.tensor_scalar(
                        out=xnt, in0=xt,
                        scalar1=ab_sb[:, bt, 0:1], scalar2=ab_sb[:, bt, 1:2],
                        op0=mybir.AluOpType.mult, op1=mybir.AluOpType.add,
                    )
                    # after xn extracted, pre-add the effective proj bias to x
                    nc.vector.tensor_scalar_add(
                        out=xt, in0=xt, scalar1=pb_sb[:, t:t + 1]
                    )
                    if debug:
                        nc.sync.dma_start(
                            out=dbg_xn[b, t * 128:(t + 1) * 128, :], in_=xnt)
                    xb.append(xt)
                    xn.append(xnt)

                # qkv: q and k in [o, n] layout (o-chunks 0-3 = q, 4-7 = k)
                qksb = []
                for i in range(8):
                    ps = psA.tile([128, N], F32, tag="big")
                    for s in range(2):
                        for t in range(CT):
                            nc.tensor.matmul(
                                out=ps[:, s * 512:(s + 1) * 512],
                                lhsT=wqkT_sb[:, t, i * 128:(i + 1) * 128],
                                rhs=xn[t][:, s * 512:(s + 1) * 512],
                                start=(t == 0), stop=(t == CT - 1),
                            )
                    qs = qk_pool.tile([128, N], BF16, tag="qk")
                    if qk_bias_zero:
                        nc.vector.tensor_copy(out=qs, in_=ps)
                    else:
                        nc.vector.tensor_scalar_add(
                            out=qs, in0=ps, scalar1=qkb_sb[:, i:i + 1]
                        )
                    if debug:
                        nc.sync.dma_start(out=dbg_qk[b, i], in_=qs)
                    qksb.append(qs)

                # v^T in [n, o] layout with an interleaved ones column:
                # vt_sb[j][p, h, 0:64] = v^T chunk, vt_sb[j][p, h, 64] = 1.0
                vtsb = []
                for j in range(NJ):
                    ps = psA.tile([128, C], F32, tag="big")
                    for t in range(CT):
                        nc.tensor.matmul(
                            out=ps,
                            lhsT=xn[t][:, j * 128:(j + 1) * 128],
                            rhs=wvT_sb[:, t, :],
                            start=(t == 0), stop=(t == CT - 1),
                        )
                    vt = vt_pool.tile([128, HEADS, CH + 1], BF16, tag="vt")
                    nc.vector.tensor_copy(
                        out=vt[:, :, 0:CH],
                        in_=ps.rearrange("p (h c) -> p h c", h=HEADS),
                    )
                    nc.vector.memset(vt[:, :, CH:CH + 1], 1.0)
                    if debug:
                        nc.sync.dma_start(out=dbg_vt[b, j], in_=vt)
                    vtsb.append(vt)

                # attention per head
                at_tiles = [
                    at_pool.tile([128, N], BF16, tag="at", name=f"at_{b}_{t}")
                    for t in range(CT)
                ]
                for h in range(HEADS):
                    qt = qksb[h // 2]
                    kt = qksb[4 + h // 2]
                    r0 = (h % 2) * CH
                    ph = psAcc.tile([CH + 1, N], F32, tag="acc")
                    for j in range(NJ):
                        sps = psA.tile([128, N], F32, tag="big")
                        for s in range(2):
                            nc.tensor.matmul(
                                out=sps[:, s * 512:(s + 1) * 512],
                                lhsT=kt[r0:r0 + CH, j * 128:(j + 1) * 128],
                                rhs=qt[r0:r0 + CH, s * 512:(s + 1) * 512],
                                start=True, stop=True,
                            )
                        esj = es_pool.tile([128, N], BF16, tag="es")
                        nc.scalar.activation(
                            out=esj, in_=sps,
                            func=mybir.ActivationFunctionType.Exp,
                            scale=SCALE,
                        )
                        if debug and h == 0:
                            nc.sync.dma_start(out=dbg_es[b, j], in_=esj)
                        for s in range(2):
                            nc.tensor.matmul(
                                out=ph[:, s * 512:(s + 1) * 512],
                                lhsT=vtsb[j][:, h, :],
                                rhs=esj[:, s * 512:(s + 1) * 512],
                                start=(j == 0), stop=(j == NJ - 1),
                            )
                    # normalize: at_h = ph[0:64] * (1/s[n]) broadcast via K=1 matmul
                    rr = rsb_pool.tile([65, N], F32, tag="rr")
                    nc.vector.reciprocal(out=rr[64:65, :], in_=ph[64:65, :])
                    rps = psA.tile([CH, N], F32, tag="big")
                    for s in range(2):
                        nc.tensor.matmul(
                            out=rps[:, s * 512:(s + 1) * 512],
                            lhsT=ones64[64:65, :],
                            rhs=rr[64:65, s * 512:(s + 1) * 512],
                            start=True, stop=True,
                        )
                    rsb = rsb_pool.tile([CH, N], F32, tag="rsb")
                    nc.vector.tensor_copy(out=rsb, in_=rps)
                    ath = ath_pool.tile([CH, N], BF16, tag="ath")
                    nc.vector.tensor_mul(out=ath, in0=ph[0:CH, :], in1=rsb)
                    # place into the head-pair stacked layout for proj
                    nc.sync.dma_start(
                        out=at_tiles[h // 2][r0:r0 + CH, :], in_=ath
                    )

                if debug:
                    for t in range(CT):
                        nc.sync.dma_start(out=dbg_at[b, t], in_=at_tiles[t])

                # proj + residual(+bias, pre-added to x)
                for i in range(CT):
                    ps = psA.tile([128, N], F32, tag="big")
                    for s in range(2):
                        for t in range(CT):
                            nc.tensor.matmul(
                                out=ps[:, s * 512:(s + 1) * 512],
                                lhsT=wpT_sb[:, t, i * 128:(i + 1) * 128],
                                rhs=at_tiles[t][:, s * 512:(s + 1) * 512],
                                start=(t == 0), stop=(t == CT - 1),
                            )
                    osb = osb_pool.tile([128, N], F32, tag="osb")
                    nc.vector.tensor_add(out=osb, in0=ps, in1=xb[i])
                    nc.sync.dma_start(
                        out=out_ext[b, i * 128:(i + 1) * 128, :], in_=osb
                    )

    nc.finalize()
    return nc


_CACHED = {}


def _get_nc(qk_bias_zero: bool) -> bass.Bass:
    key = qk_bias_zero
    if key not in _CACHED:
        _CACHED[key] = build_nc(qk_bias_zero)
    return _CACHED[key]


def _prep_inputs(x, norm_w, norm_b, qkv_w, qkv_b, proj_w, proj_b):
    bf = ml_dtypes.bfloat16
    x = np.asarray(x, dtype=np.float32).reshape(B, C, N)
    qkv_w = np.asarray(qkv_w, dtype=np.float32)
    qkv_b = np.asarray(qkv_b, dtype=np.float32)
    proj_w = np.asarray(proj_w, dtype=np.float32)
    proj_b = np.asarray(proj_b, dtype=np.float32)
    norm_w = np.asarray(norm_w, dtype=np.float32)
    norm_b = np.asarray(norm_b, dtype=np.float32)

    wqkT = np.ascontiguousarray(qkv_w[: 2 * C].T).astype(bf)          # [512, 1024]
    wvT = np.ascontiguousarray(qkv_w[2 * C:].T).astype(bf)            # [512, 512]
    wpT = np.ascontiguousarray(proj_w.T).astype(bf)                   # [512, 512]
    gnw = np.ascontiguousarray(norm_w.reshape(CT, 128).T)             # [128, 4]
    gnb = np.ascontiguousarray(norm_b.reshape(CT, 128).T)
    qkb = np.ascontiguousarray(qkv_b[: 2 * C].reshape(8, 128).T)      # [128, 8]
    pb_eff = proj_w @ qkv_b[2 * C:] + proj_b
    pbias = np.ascontiguousarray(pb_eff.reshape(CT, 128).T)           # [128, 4]

    ind = np.zeros((8, 128), np.float32)
    for g in range(8):
        ind[g, g * 16:(g + 1) * 16] = 1.0
    shared = dict(wqkT=wqkT, wvT=wvT, wpT=wpT, gnw=gnw, gnb=gnb,
                  qkb=qkb, pbias=pbias, ind=ind)
    in_maps = []
    for i in range(NCORES):
        m = dict(shared)
        m["x"] = np.ascontiguousarray(x[i * BL:(i + 1) * BL])
        in_maps.append(m)
    qk_bias_zero = bool(np.all(qkv_b[: 2 * C] == 0))
    return in_maps, qk_bias_zero


def _run(inputs, trace=False, **kw):
    in_maps, qk_bias_zero = _prep_inputs(**inputs)
    nc = _get_nc(qk_bias_zero)
    res = run_bass_kernel_spmd(nc, in_maps, list(range(NCORES)), trace=trace, **kw)
    out = np.concatenate([r["out"] for r in res.results], axis=0)
    out = out.reshape(B, C, H, W).astype(np.float32)
    return out, res


def kernel(**inputs) -> np.ndarray:
    out, _ = _run(inputs, trace=False)
    return out


if __name__ == "__main__":
    import jax

    sys.path.insert(0, "/root/problem")
    import reference

    inputs = {k: np.asarray(v) for k, v in reference.setup_inputs().items()}
    expected = np.asarray(reference.reference(**{k: jax.numpy.asarray(v) for k, v in inputs.items()}))
    actual = kernel(**inputs)
    err = np.linalg.norm(actual - expected) / np.linalg.norm(expected)
    print("Relative error:", err)
